# revision 5
# baseline (speedup 1.0000x reference)
"""Trainium2 Bass kernel v2 for LoopConnectivityDecoder.

Math: out[i,j] (i<j) = sigmoid( sum_k w2_k * relu(a[i,k] + b[j,k]) + b2 ),
symmetrized, zero diagonal; a = X@W1[:, :32].T + b1, b = X@W1[:, 32:].T.

Per (i,j) tile the kernel computes z_k = w2_k*(a_ik + b_jk) as a rank-2
outer sum on the tensor engine (K=2 bf16 matmul: lhsT=[a_k;1], rhs=[1;b_k]),
then w2_k*relu(h) = max(z,0) for w2_k>=0 / min(z,0) for w2_k<0.

Device strategy (8 cores SPMD, upper triangle in 24 (128x512) units,
3 per core; 64 k-slots sign-ordered, 16 chunks of 4 slots):
- PE: 4x row-tiled matmuls (tile_position=(32q,0)); slot 4c+q's [a;1]/[1;b]
  rows live at SBUF partitions 32q/32q+1, chunk index on the free axis.
- The 3 units' chunk streams are INTERLEAVED (u-major within each c) so
  consecutive stream slots hit different PSUM buffers and alternate drain
  engines -- this hides the matmul+semaphore handoff inside the 2-buffer
  PSUM rotation (measured ~20% on HW).
- Chunk drains split across engines per QUOTA (relu+accumulate fused):
  D: DVE scalar_tensor_tensor acc += max/min(z_psum,0) with a stride-0
     in-place accumulator (one op folds the CH k-slices, exact on HW).
  V: ScalarE relu(+-z)->fp16 staged tile, DVE fp16 tensor_tensor (2x) add.
  G: ScalarE stage, GpSimd tensor_tensor add.
  V/G adds trail the drain stream by LAG chunks to avoid FIFO inversions.
- Tail per unit: merge 3 accumulators (GpSimd+DVE), sigmoid(+b2) on
  ScalarE, DMA out.  Host folds w2/b1 into bf16 a/b row tables and
  scatters/mirrors the output.
"""

import numpy as np
import ml_dtypes

N = 1536
EMB = 32
H = 64
P = 128          # rows per unit
F = 512          # cols per unit
NCORES = 8
NBLK = N // P    # 12 row blocks
UNITS_PER_CORE = 3
CH = 4           # k-slots per chunk
NCHUNK = H // CH  # 16

# chunk drain quotas per unit (16 chunks): D = DVE fused stt,
# V = ScalarE stage + deferred DVE fp16 add, G = ScalarE stage + deferred
# GpSimd add.  Adds run at unit end so the PSUM-drain ops (STT on DVE, act
# on ScalarE) hit their engine FIFOs without cross-engine inversions.
QUOTA = {"D": 6, "V": 6, "G": 4}


def _chunk_types(npos):
    """Per-chunk drain type; the mixed-sign chunk (if any) goes to D."""
    mixed = npos // CH if npos % CH else -1
    types = [None] * NCHUNK
    counts = dict(QUOTA)
    if 0 <= mixed < NCHUNK and counts["D"] > 0:
        types[mixed] = "D"
        counts["D"] -= 1
    # largest-remainder interleave of the remaining quotas
    left = dict(counts)
    credit = {k: 0.0 for k in left}
    total = sum(left.values())
    for c in range(NCHUNK):
        if types[c] is not None:
            continue
        for k in left:
            credit[k] += counts[k] / total
        avail = [k for k in left if left[k] > 0]
        best = max(avail, key=lambda k: credit[k])
        types[c] = best
        left[best] -= 1
        credit[best] -= 1.0
    return types

_cache = {}


def _unit_list():
    """24 (row_block, col0) units covering the upper-triangle staircase."""
    units = []
    for bi in range(NBLK):
        cols = N - P * bi
        nch = -(-cols // F)
        for t in range(nch):
            col0 = min(P * bi + F * t, N - F)
            units.append((bi, col0))
    assert len(units) == NCORES * UNITS_PER_CORE
    return units


def _chunk_segments(npos):
    """Per chunk: list of (offset, width, sign) sub-ranges by w2 sign."""
    segs = []
    for c in range(NCHUNK):
        s0, s1 = c * CH, (c + 1) * CH
        if s1 <= npos:
            segs.append([(0, CH, 1.0)])
        elif s0 >= npos:
            segs.append([(0, CH, -1.0)])
        else:
            m = npos - s0
            segs.append([(0, m, 1.0), (m, CH - m, -1.0)])
    return segs


def _build_module(npos, repeat=1, ch=None, psum_bufs=2, scalar_dma=False,
                  defer_last=False, quota=None, lag=7):
    from contextlib import ExitStack
    import concourse.tile as tile
    from concourse import bacc, mybir

    global CH, NCHUNK, QUOTA
    if ch is not None:
        CH = ch
        NCHUNK = H // CH
    if quota is not None:
        QUOTA = dict(quota)
    segs = _chunk_segments(npos)
    types = _chunk_types(npos)

    nc = bacc.Bacc("TRN2", target_bir_lowering=False, debug=False,
                   num_devices=NCORES)
    A_d = nc.dram_tensor("Ag", [UNITS_PER_CORE, CH, 2, NCHUNK, P],
                         mybir.dt.bfloat16, kind="ExternalInput")
    B_d = nc.dram_tensor("Bg", [UNITS_PER_CORE, CH, 2, NCHUNK, F],
                         mybir.dt.bfloat16, kind="ExternalInput")
    b2_d = nc.dram_tensor("b2c", [P, 1], mybir.dt.float32,
                          kind="ExternalInput")
    out_d = nc.dram_tensor("out", [UNITS_PER_CORE, P, F], mybir.dt.float32,
                           kind="ExternalOutput")

    with tile.TileContext(nc) as tc, ExitStack() as ctx:
        const = ctx.enter_context(tc.tile_pool(name="const", bufs=1))
        ld = ctx.enter_context(tc.tile_pool(name="ld", bufs=1))
        stg = ctx.enter_context(tc.tile_pool(name="stg", bufs=12))
        accp = ctx.enter_context(tc.tile_pool(name="accp", bufs=1))
        outp = ctx.enter_context(tc.tile_pool(name="outp", bufs=2))
        psum = ctx.enter_context(tc.tile_pool(name="psum", bufs=psum_bufs,
                                              space="PSUM"))

        b2_t = const.tile([P, 1], mybir.dt.float32)
        nc.sync.dma_start(b2_t[:], b2_d[:])

        def body():
            # interleaved streams: chunk (u, c) for u in 0..2, c in 0..15,
            # emitted u-major within each c so consecutive stream slots hit
            # different PSUM buffers and different drain engines.
            a_ts, b_ts, accs = [], [], []
            for u in range(UNITS_PER_CORE):
                a_t = ld.tile([98, NCHUNK, P], mybir.dt.bfloat16, tag=f"a{u}")
                b_t = ld.tile([98, NCHUNK, F], mybir.dt.bfloat16, tag=f"b{u}")
                for q in range(CH):
                    nc.sync.dma_start(a_t[32 * q:32 * q + 2], A_d[u, q])
                    (nc.scalar if scalar_dma else nc.sync).dma_start(
                        b_t[32 * q:32 * q + 2], B_d[u, q])
                a_ts.append(a_t)
                b_ts.append(b_t)
                accD = accp.tile([P, 1, F], mybir.dt.float32, tag=f"accD{u}")
                accV = accp.tile([P, 1, F], mybir.dt.float16, tag=f"accV{u}")
                accG = accp.tile([P, 1, F], mybir.dt.float32, tag=f"accG{u}")
                nc.vector.memset(accD[:], 0.0)
                nc.vector.memset(accV[:], 0.0)
                nc.gpsimd.memset(accG[:], 0.0)
                accs.append((accD, accV, accG))

            # per-unit type schedule: cyclic shifts so stream neighbours use
            # different drain engines; mixed-sign chunk pinned to D.
            mixed = npos // CH if npos % CH else -1
            types_u = []
            for u in range(UNITS_PER_CORE):
                t = [types[(c + 5 * u) % NCHUNK] for c in range(NCHUNK)]
                if 0 <= mixed < NCHUNK and t[mixed] != "D":
                    j = next(i for i in range(NCHUNK) if t[i] == "D")
                    t[j], t[mixed] = t[mixed], "D"
                types_u.append(t)

            def _do_adds(u, kind, t4, seg):
                _, accV, accG = accs[u]
                for off, w, sgn in seg:
                    op1 = (mybir.AluOpType.add if sgn > 0
                           else mybir.AluOpType.subtract)
                    if kind == "V":
                        bV = accV[:].broadcast_to([P, w, F])
                        nc.vector.tensor_tensor(
                            bV, bV, t4[:, off:off + w], op1)
                    else:
                        bG = accG[:].broadcast_to([P, w, F])
                        nc.gpsimd.tensor_tensor(
                            bG, bG, t4[:, off:off + w], op1)

            LAG = lag
            pending = []  # fifo of (u, kind, t4, seg)
            for c in range(NCHUNK):
                for u in range(UNITS_PER_CORE):
                    kind = types_u[u][c]
                    y = psum.tile([P, CH, F], mybir.dt.float32, tag="y")
                    for q in range(CH):
                        nc.tensor.matmul(y[:, q],
                                         a_ts[u][32 * q:32 * q + 2, c, :],
                                         b_ts[u][32 * q:32 * q + 2, c, :],
                                         start=True, stop=True,
                                         tile_position=(32 * q, 0))
                    if kind == "D":
                        accD = accs[u][0]
                        for off, w, sgn in segs[c]:
                            op0 = (mybir.AluOpType.max if sgn > 0
                                   else mybir.AluOpType.min)
                            bD = accD[:].broadcast_to([P, w, F])
                            nc.vector.scalar_tensor_tensor(
                                bD, y[:, off:off + w], 0.0, bD,
                                op0, mybir.AluOpType.add)
                    else:
                        t4 = stg.tile([P, CH, F], mybir.dt.float16, tag="t4")
                        for off, w, sgn in segs[c]:
                            nc.scalar.activation(
                                t4[:, off:off + w], y[:, off:off + w],
                                mybir.ActivationFunctionType.Relu,
                                scale=float(sgn))
                        pending.append((u, kind, t4, segs[c]))
                    while len(pending) > LAG:
                        _do_adds(*pending.pop(0))

            for item in pending:
                _do_adds(*item)

            for u in range(UNITS_PER_CORE):
                accD, accV, accG = accs[u]
                lg = outp.tile([P, F], mybir.dt.float32, tag="lg")
                nc.gpsimd.tensor_tensor(lg[:], accD[:, 0], accV[:, 0],
                                        mybir.AluOpType.add)
                nc.vector.tensor_tensor(lg[:], lg[:], accG[:, 0],
                                        mybir.AluOpType.add)
                s_t = outp.tile([P, F], mybir.dt.float32, tag="s")
                nc.scalar.activation(s_t[:], lg[:],
                                     mybir.ActivationFunctionType.Sigmoid,
                                     bias=b2_t[:, 0:1], scale=1.0)
                nc.sync.dma_start(out_d[u], s_t[:])

        if repeat > 1:
            with tc.For_i(0, repeat, 1):
                body()
        else:
            body()

    nc.compile()
    return nc


def _prep_inputs(loop_embeddings, W1, b1, W2, b2):
    X = np.asarray(loop_embeddings, dtype=np.float32)
    W1 = np.asarray(W1, dtype=np.float32)
    b1 = np.asarray(b1, dtype=np.float32)
    W2 = np.asarray(W2, dtype=np.float32)
    b2 = np.asarray(b2, dtype=np.float32)

    a = X @ W1[:, :EMB].T + b1          # (N, H)
    bm = X @ W1[:, EMB:].T              # (N, H)
    w2 = W2[0]

    pos = np.where(w2 >= 0)[0]
    neg = np.where(w2 < 0)[0]
    order = np.concatenate([pos, neg])
    npos = len(pos)

    az = (w2[None, :] * a).T[order].astype(ml_dtypes.bfloat16)   # (H, N)
    bz = (w2[None, :] * bm).T[order].astype(ml_dtypes.bfloat16)  # (H, N)

    units = _unit_list()

    in_maps = []
    for core in range(NCORES):
        A = np.ones((UNITS_PER_CORE, CH, 2, NCHUNK, P), dtype=ml_dtypes.bfloat16)
        B = np.ones((UNITS_PER_CORE, CH, 2, NCHUNK, F), dtype=ml_dtypes.bfloat16)
        for u in range(UNITS_PER_CORE):
            bi, col0 = units[core * UNITS_PER_CORE + u]
            # slot s = CH*c + q lives at A[u, q, 0, c] / B[u, q, 1, c]
            A[u, :, 0] = az[:, bi * P:(bi + 1) * P] \
                .reshape(NCHUNK, CH, P).transpose(1, 0, 2)
            B[u, :, 1] = bz[:, col0:col0 + F] \
                .reshape(NCHUNK, CH, F).transpose(1, 0, 2)
        in_maps.append({
            "Ag": A,
            "Bg": B,
            "b2c": np.full((P, 1), b2[0], dtype=np.float32),
        })
    return in_maps, npos, units


TRACE = False
LAST_EXEC_NS = None


def kernel(loop_embeddings, W1, b1, W2, b2):
    from concourse.bass_utils import run_bass_kernel_spmd

    in_maps, npos, units = _prep_inputs(loop_embeddings, W1, b1, W2, b2)

    if npos not in _cache:
        _cache[npos] = _build_module(npos)
    nc = _cache[npos]

    res = run_bass_kernel_spmd(nc, in_maps, list(range(NCORES)))

    s = np.zeros((N, N), dtype=np.float32)
    for core in range(NCORES):
        o = res.results[core]["out"]
        for u in range(UNITS_PER_CORE):
            bi, col0 = units[core * UNITS_PER_CORE + u]
            s[bi * P:(bi + 1) * P, col0:col0 + F] = o[u]
    up = np.triu(s, 1)
    return (up + up.T).astype(np.float32)


# revision 6
# speedup vs baseline: 1.0739x; 1.0739x over previous
"""Trainium2 Bass kernel v2 for LoopConnectivityDecoder.

Math: out[i,j] (i<j) = sigmoid( sum_k w2_k * relu(a[i,k] + b[j,k]) + b2 ),
symmetrized, zero diagonal; a = X@W1[:, :32].T + b1, b = X@W1[:, 32:].T.

Per (i,j) tile the kernel computes z_k = w2_k*(a_ik + b_jk) as a rank-2
outer sum on the tensor engine (K=2 bf16 matmul: lhsT=[a_k;1], rhs=[1;b_k]),
then w2_k*relu(h) = max(z,0) for w2_k>=0 / min(z,0) for w2_k<0.

Device strategy (8 cores SPMD, upper triangle in 24 (128x512) units,
3 per core; 64 k-slots sign-ordered, 16 chunks of 4 slots):
- PE: 4x row-tiled matmuls (tile_position=(32q,0)); slot 4c+q's [a;1]/[1;b]
  rows live at SBUF partitions 32q/32q+1, chunk index on the free axis.
- The 3 units' chunk streams are INTERLEAVED (u-major within each c) so
  consecutive stream slots hit different PSUM buffers and alternate drain
  engines -- this hides the matmul+semaphore handoff inside the 2-buffer
  PSUM rotation (measured ~20% on HW).
- Chunk drains split across engines per QUOTA (relu+accumulate fused):
  D: DVE scalar_tensor_tensor acc += max/min(z_psum,0) with a stride-0
     in-place accumulator (one op folds the CH k-slices, exact on HW).
  V: ScalarE relu(+-z)->fp16 staged tile, DVE fp16 tensor_tensor (2x) add.
  G: ScalarE stage, GpSimd tensor_tensor add.
  V/G adds trail the drain stream by LAG chunks to avoid FIFO inversions.
- Tail per unit: merge 3 accumulators (GpSimd+DVE), sigmoid(+b2) on
  ScalarE, DMA out.  Host folds w2/b1 into bf16 a/b row tables and
  scatters/mirrors the output.
"""

import numpy as np
import ml_dtypes

N = 1536
EMB = 32
H = 64
P = 128          # rows per unit
F = 512          # cols per unit
NCORES = 8
NBLK = N // P    # 12 row blocks
UNITS_PER_CORE = 3
CH = 4           # k-slots per chunk
NCHUNK = H // CH  # 16

# chunk drain quotas per unit (16 chunks): D = DVE fused stt,
# V = ScalarE stage + deferred DVE fp16 add, G = ScalarE stage + deferred
# GpSimd add.  Adds run at unit end so the PSUM-drain ops (STT on DVE, act
# on ScalarE) hit their engine FIFOs without cross-engine inversions.
QUOTA = {"D": 6, "V": 6, "G": 4}


def _chunk_types(npos):
    """Per-chunk drain type; the mixed-sign chunk (if any) goes to D."""
    mixed = npos // CH if npos % CH else -1
    types = [None] * NCHUNK
    counts = dict(QUOTA)
    if 0 <= mixed < NCHUNK and counts["D"] > 0:
        types[mixed] = "D"
        counts["D"] -= 1
    # largest-remainder interleave of the remaining quotas
    left = dict(counts)
    credit = {k: 0.0 for k in left}
    total = sum(left.values())
    for c in range(NCHUNK):
        if types[c] is not None:
            continue
        for k in left:
            credit[k] += counts[k] / total
        avail = [k for k in left if left[k] > 0]
        best = max(avail, key=lambda k: credit[k])
        types[c] = best
        left[best] -= 1
        credit[best] -= 1.0
    return types

_cache = {}


def _unit_list():
    """24 (row_block, col0) units covering the upper-triangle staircase."""
    units = []
    for bi in range(NBLK):
        cols = N - P * bi
        nch = -(-cols // F)
        for t in range(nch):
            col0 = min(P * bi + F * t, N - F)
            units.append((bi, col0))
    assert len(units) == NCORES * UNITS_PER_CORE
    return units


def _chunk_segments(npos):
    """Per chunk: list of (offset, width, sign) sub-ranges by w2 sign."""
    segs = []
    for c in range(NCHUNK):
        s0, s1 = c * CH, (c + 1) * CH
        if s1 <= npos:
            segs.append([(0, CH, 1.0)])
        elif s0 >= npos:
            segs.append([(0, CH, -1.0)])
        else:
            m = npos - s0
            segs.append([(0, m, 1.0), (m, CH - m, -1.0)])
    return segs


def _build_module(npos, repeat=1, ch=None, psum_bufs=2, scalar_dma=False,
                  defer_last=False, quota=None, lag=9):
    from contextlib import ExitStack
    import concourse.tile as tile
    from concourse import bacc, mybir

    global CH, NCHUNK, QUOTA
    if ch is not None:
        CH = ch
        NCHUNK = H // CH
    if quota is not None:
        QUOTA = dict(quota)
    segs = _chunk_segments(npos)
    types = _chunk_types(npos)

    nc = bacc.Bacc("TRN2", target_bir_lowering=False, debug=False,
                   num_devices=NCORES)
    A_d = nc.dram_tensor("Ag", [UNITS_PER_CORE, CH, 2, NCHUNK, P],
                         mybir.dt.bfloat16, kind="ExternalInput")
    B_d = nc.dram_tensor("Bg", [UNITS_PER_CORE, CH, 2, NCHUNK, F],
                         mybir.dt.bfloat16, kind="ExternalInput")
    b2_d = nc.dram_tensor("b2c", [P, 1], mybir.dt.float32,
                          kind="ExternalInput")
    out_d = nc.dram_tensor("out", [UNITS_PER_CORE, P, F], mybir.dt.float32,
                           kind="ExternalOutput")

    with tile.TileContext(nc) as tc, ExitStack() as ctx:
        const = ctx.enter_context(tc.tile_pool(name="const", bufs=1))
        ld = ctx.enter_context(tc.tile_pool(name="ld", bufs=1))
        stg = ctx.enter_context(tc.tile_pool(name="stg", bufs=12))
        accp = ctx.enter_context(tc.tile_pool(name="accp", bufs=1))
        outp = ctx.enter_context(tc.tile_pool(name="outp", bufs=3))
        psum = ctx.enter_context(tc.tile_pool(name="psum", bufs=psum_bufs,
                                              space="PSUM"))

        b2_t = const.tile([P, 1], mybir.dt.float32)
        nc.sync.dma_start(b2_t[:], b2_d[:])

        def body():
            # interleaved streams: chunk (u, c) for u in 0..2, c in 0..15,
            # emitted u-major within each c so consecutive stream slots hit
            # different PSUM buffers and different drain engines.
            a_ts, b_ts, accs = [], [], []
            for u in range(UNITS_PER_CORE):
                a_t = ld.tile([98, NCHUNK, P], mybir.dt.bfloat16, tag=f"a{u}")
                b_t = ld.tile([98, NCHUNK, F], mybir.dt.bfloat16, tag=f"b{u}")
                for q in range(CH):
                    nc.sync.dma_start(a_t[32 * q:32 * q + 2], A_d[u, q])
                    (nc.scalar if scalar_dma else nc.sync).dma_start(
                        b_t[32 * q:32 * q + 2], B_d[u, q])
                a_ts.append(a_t)
                b_ts.append(b_t)
                accD = accp.tile([P, 1, F], mybir.dt.float32, tag=f"accD{u}")
                accV = accp.tile([P, 1, F], mybir.dt.float16, tag=f"accV{u}")
                accG = accp.tile([P, 1, F], mybir.dt.float32, tag=f"accG{u}")
                nc.vector.memset(accD[:], 0.0)
                nc.vector.memset(accV[:], 0.0)
                nc.gpsimd.memset(accG[:], 0.0)
                accs.append((accD, accV, accG))

            # per-unit type schedule: cyclic shifts so stream neighbours use
            # different drain engines; mixed-sign chunk pinned to D.
            mixed = npos // CH if npos % CH else -1
            types_u = []
            for u in range(UNITS_PER_CORE):
                t = [types[(c + 5 * u) % NCHUNK] for c in range(NCHUNK)]
                if 0 <= mixed < NCHUNK and t[mixed] != "D":
                    j = next(i for i in range(NCHUNK) if t[i] == "D")
                    t[j], t[mixed] = t[mixed], "D"
                types_u.append(t)

            def _do_adds(u, kind, t4, seg):
                _, accV, accG = accs[u]
                for off, w, sgn in seg:
                    op1 = (mybir.AluOpType.add if sgn > 0
                           else mybir.AluOpType.subtract)
                    if kind == "V":
                        bV = accV[:].broadcast_to([P, w, F])
                        nc.vector.tensor_tensor(
                            bV, bV, t4[:, off:off + w], op1)
                    else:
                        bG = accG[:].broadcast_to([P, w, F])
                        nc.gpsimd.tensor_tensor(
                            bG, bG, t4[:, off:off + w], op1)

            LAG = lag
            pending = []  # fifo of (u, kind, t4, seg)
            for c in range(NCHUNK):
                for u in range(UNITS_PER_CORE):
                    kind = types_u[u][c]
                    y = psum.tile([P, CH, F], mybir.dt.float32, tag="y")
                    for q in range(CH):
                        nc.tensor.matmul(y[:, q],
                                         a_ts[u][32 * q:32 * q + 2, c, :],
                                         b_ts[u][32 * q:32 * q + 2, c, :],
                                         start=True, stop=True,
                                         tile_position=(32 * q, 0))
                    if kind == "D":
                        accD = accs[u][0]
                        for off, w, sgn in segs[c]:
                            op0 = (mybir.AluOpType.max if sgn > 0
                                   else mybir.AluOpType.min)
                            bD = accD[:].broadcast_to([P, w, F])
                            nc.vector.scalar_tensor_tensor(
                                bD, y[:, off:off + w], 0.0, bD,
                                op0, mybir.AluOpType.add)
                    else:
                        t4 = stg.tile([P, CH, F], mybir.dt.float16, tag="t4")
                        for off, w, sgn in segs[c]:
                            nc.scalar.activation(
                                t4[:, off:off + w], y[:, off:off + w],
                                mybir.ActivationFunctionType.Relu,
                                scale=float(sgn))
                        pending.append((u, kind, t4, segs[c]))
                    while len(pending) > LAG:
                        _do_adds(*pending.pop(0))

            for item in pending:
                _do_adds(*item)

            for u in range(UNITS_PER_CORE):
                accD, accV, accG = accs[u]
                lg = outp.tile([P, F], mybir.dt.float32, tag="lg")
                nc.gpsimd.tensor_tensor(lg[:], accD[:, 0], accV[:, 0],
                                        mybir.AluOpType.add)
                nc.vector.tensor_tensor(lg[:], lg[:], accG[:, 0],
                                        mybir.AluOpType.add)
                s_t = outp.tile([P, F], mybir.dt.float32, tag="s")
                nc.scalar.activation(s_t[:], lg[:],
                                     mybir.ActivationFunctionType.Sigmoid,
                                     bias=b2_t[:, 0:1], scale=1.0)
                nc.sync.dma_start(out_d[u], s_t[:])

        if repeat > 1:
            with tc.For_i(0, repeat, 1):
                body()
        else:
            body()

    nc.compile()
    return nc


def _prep_inputs(loop_embeddings, W1, b1, W2, b2):
    X = np.asarray(loop_embeddings, dtype=np.float32)
    W1 = np.asarray(W1, dtype=np.float32)
    b1 = np.asarray(b1, dtype=np.float32)
    W2 = np.asarray(W2, dtype=np.float32)
    b2 = np.asarray(b2, dtype=np.float32)

    a = X @ W1[:, :EMB].T + b1          # (N, H)
    bm = X @ W1[:, EMB:].T              # (N, H)
    w2 = W2[0]

    pos = np.where(w2 >= 0)[0]
    neg = np.where(w2 < 0)[0]
    order = np.concatenate([pos, neg])
    npos = len(pos)

    az = (w2[None, :] * a).T[order].astype(ml_dtypes.bfloat16)   # (H, N)
    bz = (w2[None, :] * bm).T[order].astype(ml_dtypes.bfloat16)  # (H, N)

    units = _unit_list()

    in_maps = []
    for core in range(NCORES):
        A = np.ones((UNITS_PER_CORE, CH, 2, NCHUNK, P), dtype=ml_dtypes.bfloat16)
        B = np.ones((UNITS_PER_CORE, CH, 2, NCHUNK, F), dtype=ml_dtypes.bfloat16)
        for u in range(UNITS_PER_CORE):
            bi, col0 = units[core * UNITS_PER_CORE + u]
            # slot s = CH*c + q lives at A[u, q, 0, c] / B[u, q, 1, c]
            A[u, :, 0] = az[:, bi * P:(bi + 1) * P] \
                .reshape(NCHUNK, CH, P).transpose(1, 0, 2)
            B[u, :, 1] = bz[:, col0:col0 + F] \
                .reshape(NCHUNK, CH, F).transpose(1, 0, 2)
        in_maps.append({
            "Ag": A,
            "Bg": B,
            "b2c": np.full((P, 1), b2[0], dtype=np.float32),
        })
    return in_maps, npos, units


TRACE = False
LAST_EXEC_NS = None


def kernel(loop_embeddings, W1, b1, W2, b2):
    from concourse.bass_utils import run_bass_kernel_spmd

    in_maps, npos, units = _prep_inputs(loop_embeddings, W1, b1, W2, b2)

    if npos not in _cache:
        _cache[npos] = _build_module(npos)
    nc = _cache[npos]

    res = run_bass_kernel_spmd(nc, in_maps, list(range(NCORES)))

    s = np.zeros((N, N), dtype=np.float32)
    for core in range(NCORES):
        o = res.results[core]["out"]
        for u in range(UNITS_PER_CORE):
            bi, col0 = units[core * UNITS_PER_CORE + u]
            s[bi * P:(bi + 1) * P, col0:col0 + F] = o[u]
    up = np.triu(s, 1)
    return (up + up.T).astype(np.float32)


# revision 7
# speedup vs baseline: 1.1066x; 1.0305x over previous
"""Trainium2 Bass kernel v2 for LoopConnectivityDecoder.

Math: out[i,j] (i<j) = sigmoid( sum_k w2_k * relu(a[i,k] + b[j,k]) + b2 ),
symmetrized, zero diagonal; a = X@W1[:, :32].T + b1, b = X@W1[:, 32:].T.

Per (i,j) tile the kernel computes z_k = w2_k*(a_ik + b_jk) as a rank-2
outer sum on the tensor engine (K=2 bf16 matmul: lhsT=[a_k;1], rhs=[1;b_k]),
then w2_k*relu(h) = max(z,0) for w2_k>=0 / min(z,0) for w2_k<0.

Device strategy (8 cores SPMD, upper triangle in 24 (128x512) units,
3 per core; 64 k-slots sign-ordered, 16 chunks of 4 slots):
- PE: 4x row-tiled matmuls (tile_position=(32q,0)); slot 4c+q's [a;1]/[1;b]
  rows live at SBUF partitions 32q/32q+1, chunk index on the free axis.
- The 3 units' chunk streams are INTERLEAVED (u-major within each c) so
  consecutive stream slots hit different PSUM buffers and alternate drain
  engines -- this hides the matmul+semaphore handoff inside the 2-buffer
  PSUM rotation (measured ~20% on HW).
- Chunk drains split across engines per QUOTA (relu+accumulate fused):
  D: DVE scalar_tensor_tensor acc += max/min(z_psum,0) with a stride-0
     in-place accumulator (one op folds the CH k-slices, exact on HW).
  V: ScalarE relu(+-z)->fp16 staged tile, DVE fp16 tensor_tensor (2x) add.
  G: ScalarE stage, GpSimd tensor_tensor add.
  V/G adds trail the drain stream by LAG chunks to avoid FIFO inversions.
- Tail per unit: merge 3 accumulators (GpSimd+DVE), sigmoid(+b2) on
  ScalarE, DMA out.  Host folds w2/b1 into bf16 a/b row tables and
  scatters/mirrors the output.
"""

import numpy as np
import ml_dtypes

N = 1536
EMB = 32
H = 64
P = 128          # rows per unit
F = 512          # cols per unit
NCORES = 8
NBLK = N // P    # 12 row blocks
UNITS_PER_CORE = 3
CH = 4           # k-slots per chunk
NCHUNK = H // CH  # 16

# chunk drain quotas per unit (16 chunks): D = DVE fused stt,
# V = ScalarE stage + deferred DVE fp16 add, G = ScalarE stage + deferred
# GpSimd add.  Adds run at unit end so the PSUM-drain ops (STT on DVE, act
# on ScalarE) hit their engine FIFOs without cross-engine inversions.
QUOTA = {"D": 6, "V": 6, "G": 4}


def _chunk_types(npos):
    """Per-chunk drain type; the mixed-sign chunk (if any) goes to D."""
    mixed = npos // CH if npos % CH else -1
    types = [None] * NCHUNK
    counts = dict(QUOTA)
    if 0 <= mixed < NCHUNK and counts["D"] > 0:
        types[mixed] = "D"
        counts["D"] -= 1
    # largest-remainder interleave of the remaining quotas
    left = dict(counts)
    credit = {k: 0.0 for k in left}
    total = sum(left.values())
    for c in range(NCHUNK):
        if types[c] is not None:
            continue
        for k in left:
            credit[k] += counts[k] / total
        avail = [k for k in left if left[k] > 0]
        best = max(avail, key=lambda k: credit[k])
        types[c] = best
        left[best] -= 1
        credit[best] -= 1.0
    return types

_cache = {}


def _unit_list():
    """24 (row_block, col0) units covering the upper-triangle staircase."""
    units = []
    for bi in range(NBLK):
        cols = N - P * bi
        nch = -(-cols // F)
        for t in range(nch):
            col0 = min(P * bi + F * t, N - F)
            units.append((bi, col0))
    assert len(units) == NCORES * UNITS_PER_CORE
    return units


def _chunk_segments(npos):
    """Per chunk: list of (offset, width, sign) sub-ranges by w2 sign."""
    segs = []
    for c in range(NCHUNK):
        s0, s1 = c * CH, (c + 1) * CH
        if s1 <= npos:
            segs.append([(0, CH, 1.0)])
        elif s0 >= npos:
            segs.append([(0, CH, -1.0)])
        else:
            m = npos - s0
            segs.append([(0, m, 1.0), (m, CH - m, -1.0)])
    return segs


def _build_module(npos, repeat=1, ch=None, psum_bufs=2, scalar_dma=False,
                  defer_last=False, quota=None, lag=9):
    from contextlib import ExitStack
    import concourse.tile as tile
    from concourse import bacc, mybir

    global CH, NCHUNK, QUOTA
    if ch is not None:
        CH = ch
        NCHUNK = H // CH
    if quota is not None:
        QUOTA = dict(quota)
    segs = _chunk_segments(npos)
    types = _chunk_types(npos)

    nc = bacc.Bacc("TRN2", target_bir_lowering=False, debug=False,
                   num_devices=NCORES)
    A_d = nc.dram_tensor("Ag", [UNITS_PER_CORE, CH, 2, NCHUNK, P],
                         mybir.dt.bfloat16, kind="ExternalInput")
    B_d = nc.dram_tensor("Bg", [UNITS_PER_CORE, CH, 2, NCHUNK, F],
                         mybir.dt.bfloat16, kind="ExternalInput")
    b2_d = nc.dram_tensor("b2c", [P, 1], mybir.dt.float32,
                          kind="ExternalInput")
    out_d = nc.dram_tensor("out", [UNITS_PER_CORE, P, F], mybir.dt.float32,
                           kind="ExternalOutput")

    with tile.TileContext(nc) as tc, ExitStack() as ctx:
        const = ctx.enter_context(tc.tile_pool(name="const", bufs=1))
        ld = ctx.enter_context(tc.tile_pool(name="ld", bufs=1))
        stg = ctx.enter_context(tc.tile_pool(name="stg", bufs=12))
        accp = ctx.enter_context(tc.tile_pool(name="accp", bufs=1))
        outp = ctx.enter_context(tc.tile_pool(name="outp", bufs=3))
        psum = ctx.enter_context(tc.tile_pool(name="psum", bufs=psum_bufs,
                                              space="PSUM"))

        b2_t = const.tile([P, 1], mybir.dt.float32)
        nc.sync.dma_start(b2_t[:], b2_d[:])

        def body():
            # interleaved streams: chunk (u, c) for u in 0..2, c in 0..15,
            # emitted u-major within each c so consecutive stream slots hit
            # different PSUM buffers and different drain engines.
            a_ts, b_ts, accs = [], [], []
            for u in range(UNITS_PER_CORE):
                a_t = ld.tile([98, NCHUNK, P], mybir.dt.bfloat16, tag=f"a{u}")
                b_t = ld.tile([98, NCHUNK, F], mybir.dt.bfloat16, tag=f"b{u}")
                for q in range(CH):
                    nc.sync.dma_start(a_t[32 * q:32 * q + 2], A_d[u, q])
                    (nc.scalar if scalar_dma else nc.sync).dma_start(
                        b_t[32 * q:32 * q + 2], B_d[u, q])
                a_ts.append(a_t)
                b_ts.append(b_t)
                accD = accp.tile([P, 1, F], mybir.dt.float32, tag=f"accD{u}")
                accV = accp.tile([P, 1, F], mybir.dt.float16, tag=f"accV{u}")
                accG = accp.tile([P, 1, F], mybir.dt.float32, tag=f"accG{u}")
                nc.vector.memset(accD[:], 0.0)
                nc.vector.memset(accV[:], 0.0)
                nc.gpsimd.memset(accG[:], 0.0)
                accs.append((accD, accV, accG))

            # per-unit type schedule: cyclic shifts so stream neighbours use
            # different drain engines; mixed-sign chunk pinned to D.
            mixed = npos // CH if npos % CH else -1
            types_u = []
            for u in range(UNITS_PER_CORE):
                t = [types[(c + 5 * u) % NCHUNK] for c in range(NCHUNK)]
                if 0 <= mixed < NCHUNK and t[mixed] != "D":
                    j = next(i for i in range(NCHUNK) if t[i] == "D")
                    t[j], t[mixed] = t[mixed], "D"
                types_u.append(t)

            def _do_adds(u, kind, t4, seg):
                _, accV, accG = accs[u]
                for off, w, sgn in seg:
                    op1 = (mybir.AluOpType.add if sgn > 0
                           else mybir.AluOpType.subtract)
                    if kind == "V":
                        bV = accV[:].broadcast_to([P, w, F])
                        nc.vector.tensor_tensor(
                            bV, bV, t4[:, off:off + w], op1)
                    else:
                        bG = accG[:].broadcast_to([P, w, F])
                        nc.gpsimd.tensor_tensor(
                            bG, bG, t4[:, off:off + w], op1)

            LAG = lag
            pending = []  # fifo of (u, kind, t4, seg)

            def _tail(u):
                w = F
                accD, accV, accG = accs[u]
                lg = outp.tile([P, F], mybir.dt.float32, tag="lg")
                nc.gpsimd.tensor_tensor(lg[:], accD[:, 0], accV[:, 0],
                                        mybir.AluOpType.add)
                nc.vector.tensor_tensor(lg[:], lg[:], accG[:, 0],
                                        mybir.AluOpType.add)
                s_t = outp.tile([P, F], mybir.dt.float32, tag="s")
                nc.scalar.activation(s_t[:], lg[:],
                                     mybir.ActivationFunctionType.Sigmoid,
                                     bias=b2_t[:, 0:1], scale=1.0)
                nc.sync.dma_start(out_d[u], s_t[:])

            # skewed interleave: unit u runs u chunks behind unit 0 so units
            # finish staggered; each unit's tail overlaps the others' chunks.
            stream = []
            for r in range(NCHUNK + UNITS_PER_CORE - 1):
                for u in range(UNITS_PER_CORE):
                    cc = r - u
                    if 0 <= cc < NCHUNK:
                        stream.append((u, cc))
            for u, c in stream:
                kind = types_u[u][c]
                y = psum.tile([P, CH, F], mybir.dt.float32, tag="y")
                for q in range(CH):
                    nc.tensor.matmul(y[:, q],
                                     a_ts[u][32 * q:32 * q + 2, c, :],
                                     b_ts[u][32 * q:32 * q + 2, c, :],
                                     start=True, stop=True,
                                     tile_position=(32 * q, 0))
                if kind == "D":
                    accD = accs[u][0]
                    for off, w, sgn in segs[c]:
                        op0 = (mybir.AluOpType.max if sgn > 0
                               else mybir.AluOpType.min)
                        bD = accD[:].broadcast_to([P, w, F])
                        nc.vector.scalar_tensor_tensor(
                            bD, y[:, off:off + w], 0.0, bD,
                            op0, mybir.AluOpType.add)
                else:
                    t4 = stg.tile([P, CH, F], mybir.dt.float16, tag="t4")
                    for off, w, sgn in segs[c]:
                        nc.scalar.activation(
                            t4[:, off:off + w], y[:, off:off + w],
                            mybir.ActivationFunctionType.Relu,
                            scale=float(sgn))
                    pending.append((u, kind, t4, segs[c]))
                while len(pending) > LAG:
                    _do_adds(*pending.pop(0))
                if c == NCHUNK - 1:
                    rest = [p for p in pending if p[0] == u]
                    pending = [p for p in pending if p[0] != u]
                    for p in rest:
                        _do_adds(*p)
                    _tail(u)

        if repeat > 1:
            with tc.For_i(0, repeat, 1):
                body()
        else:
            body()

    nc.compile()
    return nc


def _prep_inputs(loop_embeddings, W1, b1, W2, b2):
    X = np.asarray(loop_embeddings, dtype=np.float32)
    W1 = np.asarray(W1, dtype=np.float32)
    b1 = np.asarray(b1, dtype=np.float32)
    W2 = np.asarray(W2, dtype=np.float32)
    b2 = np.asarray(b2, dtype=np.float32)

    a = X @ W1[:, :EMB].T + b1          # (N, H)
    bm = X @ W1[:, EMB:].T              # (N, H)
    w2 = W2[0]

    pos = np.where(w2 >= 0)[0]
    neg = np.where(w2 < 0)[0]
    order = np.concatenate([pos, neg])
    npos = len(pos)

    az = (w2[None, :] * a).T[order].astype(ml_dtypes.bfloat16)   # (H, N)
    bz = (w2[None, :] * bm).T[order].astype(ml_dtypes.bfloat16)  # (H, N)

    units = _unit_list()

    in_maps = []
    for core in range(NCORES):
        A = np.ones((UNITS_PER_CORE, CH, 2, NCHUNK, P), dtype=ml_dtypes.bfloat16)
        B = np.ones((UNITS_PER_CORE, CH, 2, NCHUNK, F), dtype=ml_dtypes.bfloat16)
        for u in range(UNITS_PER_CORE):
            bi, col0 = units[core * UNITS_PER_CORE + u]
            # slot s = CH*c + q lives at A[u, q, 0, c] / B[u, q, 1, c]
            A[u, :, 0] = az[:, bi * P:(bi + 1) * P] \
                .reshape(NCHUNK, CH, P).transpose(1, 0, 2)
            B[u, :, 1] = bz[:, col0:col0 + F] \
                .reshape(NCHUNK, CH, F).transpose(1, 0, 2)
        in_maps.append({
            "Ag": A,
            "Bg": B,
            "b2c": np.full((P, 1), b2[0], dtype=np.float32),
        })
    return in_maps, npos, units


TRACE = False
LAST_EXEC_NS = None


def kernel(loop_embeddings, W1, b1, W2, b2):
    from concourse.bass_utils import run_bass_kernel_spmd

    in_maps, npos, units = _prep_inputs(loop_embeddings, W1, b1, W2, b2)

    if npos not in _cache:
        _cache[npos] = _build_module(npos)
    nc = _cache[npos]

    res = run_bass_kernel_spmd(nc, in_maps, list(range(NCORES)))

    s = np.zeros((N, N), dtype=np.float32)
    for core in range(NCORES):
        o = res.results[core]["out"]
        for u in range(UNITS_PER_CORE):
            bi, col0 = units[core * UNITS_PER_CORE + u]
            s[bi * P:(bi + 1) * P, col0:col0 + F] = o[u]
    up = np.triu(s, 1)
    return (up + up.T).astype(np.float32)
